# revision 19
# baseline (speedup 1.0000x reference)
"""AttentionBlock1D Trainium2 kernel.

Reference computation (per batch element b):
    xn   = GroupNorm(x, 8 groups, eps=1e-5) * gamma + beta
    qkv  = w_qkv @ xn                  # [3C, S], 1x1 conv == channel matmul
    q,k,v split; heads H=8, D=64
    att  = softmax(q^T k / sqrt(D)) v  # per head
    out  = w_out @ att + b_out + x

Distribution: pure data parallelism — batch B=8, one batch element per
NeuronCore (8 cores). Weights replicated.

Per-core dataflow (matmuls in float32r [tf32-like, full PE rate at
N>=256]; the AV stage runs bf16; fp32 PSUM accumulation everywhere):
    - GroupNorm stats via bn_stats/bn_aggr per channel; group reduce and
      broadcast-back are tiny masked matmuls (cross-partition on PE).
    - QKV keeps q,k in [channel, seq] layout; v is produced transposed
      ([seq, channel]) by swapping matmul operands, with a ones column
      appended per head (softmax denominator trick).
    - scores^T[t,s] = (k_h)^T q_h computed directly in [t, s] layout so
      softmax's exp reads PSUM and the AV matmul needs no transposes.
      Max-subtraction is skipped (scores are O(5); exp safe in fp32),
      mathematically identical to jax.nn.softmax.
    - exp on ScalarE (critical engine: S*S*H = 8.4M activations),
      emitted two heads ahead of the AV consumer.
    - AV accumulates [d, s] plus a denominator row over t chunks; the
      normalization multiplies by a K=1 broadcast matmul of the
      reciprocal denominator during PSUM evacuation.
    - output projection accumulates over channel chunks and adds
      (x + b_out) during evacuation.
"""

import numpy as np

B, C, S = 8, 512, 1024
H = 8            # heads
D = C // H       # 64 head dim
EPS = 1e-5
CCH = C // 128   # 4 channel chunks of 128
TT = S // 128    # 8 t (key) chunks of 128
NS = 512         # matmul moving free dim
SH = S // NS     # 2 s halves

_CACHE = {}


def _build(e_bufs=3):
    import concourse.bacc as bacc
    import concourse.tile as tile
    import concourse.mybir as mybir

    nc = bacc.Bacc("TRN2", target_bir_lowering=False, debug=False)
    R = mybir.dt.float32r
    F = mybir.dt.float32

    x_d = nc.dram_tensor("x", [C, S], R, kind="ExternalInput").ap()
    wqkvT_d = nc.dram_tensor("wqkvT", [C, 3 * C], R, kind="ExternalInput").ap()
    woutT_d = nc.dram_tensor("woutT", [C, C], R, kind="ExternalInput").ap()
    gamma_d = nc.dram_tensor("gamma", [C], F, kind="ExternalInput").ap()
    beta_d = nc.dram_tensor("beta", [C], F, kind="ExternalInput").ap()
    bout_d = nc.dram_tensor("bout", [C], F, kind="ExternalInput").ap()
    gmask_d = nc.dram_tensor("gmask", [128, 2], R, kind="ExternalInput").ap()
    bmask_d = nc.dram_tensor("bmask", [2, 128], R, kind="ExternalInput").ap()
    onesr_d = nc.dram_tensor("onesr", [1, 64], R, kind="ExternalInput").ap()
    ident_d = nc.dram_tensor("ident", [128, 128], R, kind="ExternalInput").ap()
    onesv_d = nc.dram_tensor("onesv", [128, 64], mybir.dt.bfloat16,
                             kind="ExternalInput").ap()
    out_d = nc.dram_tensor("out", [C, S], R, kind="ExternalOutput").ap()

    with tile.TileContext(nc) as tc:
        _body(tc, nc, mybir, x_d, wqkvT_d, woutT_d, gamma_d, beta_d, bout_d,
              gmask_d, bmask_d, onesr_d, onesv_d, ident_d, out_d, e_bufs)
    nc.compile()
    return nc


def _body(tc, nc, mybir, x_d, wqkvT_d, woutT_d, gamma_d, beta_d, bout_d,
          gmask_d, bmask_d, onesr_d, onesv_d, ident_d, out_d, e_bufs):
    from contextlib import ExitStack

    R = mybir.dt.float32r
    F = mybir.dt.float32
    BF = mybir.dt.bfloat16
    OP = mybir.AluOpType
    AF = mybir.ActivationFunctionType

    ctx = ExitStack()
    with ctx:
        const = ctx.enter_context(tc.tile_pool(name="const", bufs=1))
        small = ctx.enter_context(tc.tile_pool(name="small", bufs=4))
        rpool = ctx.enter_context(tc.tile_pool(name="rpool", bufs=2))
        dpool = ctx.enter_context(tc.tile_pool(name="dpool", bufs=2, space="DRAM"))
        epool = ctx.enter_context(tc.tile_pool(name="epool", bufs=4))
        evac = ctx.enter_context(tc.tile_pool(name="evac", bufs=2))

        # ---- resident tensors ----
        x_sb = const.tile([128, CCH, S], R, tag="x")        # x, later x + b_out
        wq_sb = const.tile([128, CCH, 3 * C], R, tag="wq")
        wo_sb = const.tile([128, CCH, C], R, tag="wo")
        gamma_sb = const.tile([128, CCH], F, tag="gam")
        beta_sb = const.tile([128, CCH], F, tag="bet")
        bout_sb = const.tile([128, CCH], F, tag="bou")
        qk_sb = const.tile([128, 2 * CCH, S], R, tag="qk")  # q: 0-3, k: 4-7
        vT_sb = const.tile([128, TT, H, D + 1], BF, tag="vt")  # +ones column
        # xn shares space with att (xn dead after QKV, att written after)
        xn_sb = const.tile([128, CCH, S], R, tag="xn_att")
        att_sb = None  # allocated later from the same tag

        for c in range(CCH):
            nc.sync.dma_start(out=x_sb[:, c, :],
                              in_=x_d.rearrange("(c p) s -> p c s", p=128)[:, c, :])
        nc.sync.dma_start(out=gamma_sb, in_=gamma_d.rearrange("(c p) -> p c", p=128))
        nc.sync.dma_start(out=beta_sb, in_=beta_d.rearrange("(c p) -> p c", p=128))
        nc.sync.dma_start(out=bout_sb, in_=bout_d.rearrange("(c p) -> p c", p=128))
        for c in range(CCH):
            nc.sync.dma_start(out=wq_sb[:, c, :],
                              in_=wqkvT_d.rearrange("(c p) o -> p c o", p=128)[:, c, :])

        gmask = const.tile([128, 2], R, tag="gmask")   # [channel, group] 1/64
        bmask = const.tile([2, 128], R, tag="bmask")   # [group, channel] 1
        ones_row = const.tile([1, 64], R, tag="ones")  # K=1 broadcast stationary
        ident_sb = const.tile([128, 128], R, tag="ident")
        nc.sync.dma_start(out=gmask, in_=gmask_d)
        nc.sync.dma_start(out=bmask, in_=bmask_d)
        nc.sync.dma_start(out=ones_row, in_=onesr_d)
        nc.sync.dma_start(out=ident_sb, in_=ident_d)
        nc.sync.dma_start(
            out=vT_sb[:, :, :, D:D + 1],
            in_=onesv_d.rearrange("p (t h) -> p t h", t=TT)[:, :, :, None])

        # scores/exp psum pool lives from the start (banks 0-3, LIFO inside)
        ps_sc_pool = ctx.enter_context(
            tc.tile_pool(name="ps_sc", bufs=2, space="PSUM"))

        # =========================== GroupNorm ===========================
        # Stage-batched across the 4 channel chunks to minimize
        # cross-engine ping-pong: one group-reduce matmul and one
        # broadcast-back matmul handle all 8 groups at once.
        with tc.tile_pool(name="ps_gn", bufs=2, space="PSUM") as ps_gn:
            st = small.tile([128, CCH, 2, 6], F, tag="gn_st")
            for c in range(CCH):
                nc.vector.bn_stats(st[:, c, 0, :], x_sb[:, c, 0:512])
                nc.vector.bn_stats(st[:, c, 1, :], x_sb[:, c, 512:1024])
            mv = small.tile([128, CCH, 2], F, tag="gn_mv")  # per-ch mean, var
            for c in range(CCH):
                nc.vector.bn_aggr(mv[:, c, :], st[:, c, :, :])
            ms = small.tile([128, CCH, 2], R, tag="gn_ms")  # mean, E[x^2]
            nc.vector.tensor_copy(ms[:, :, 0:1], mv[:, :, 0:1])
            nc.vector.tensor_mul(ms[:, :, 1:2], mv[:, :, 0:1], mv[:, :, 0:1])
            nc.vector.tensor_add(ms[:, :, 1:2], ms[:, :, 1:2], mv[:, :, 1:2])
            # one group reduce for all chunks: [2, (chunk, stat)]
            gs_ps = ps_gn.tile([2, 2 * CCH], F, tag="gn_ps")
            nc.tensor.matmul(gs_ps[:], gmask[:],
                             ms[:].rearrange("p c s -> p (c s)"),
                             start=True, stop=True)
            gs = small.tile([2, CCH, 2], F, tag="gn_gs")
            nc.vector.tensor_copy(gs[:].rearrange("p c s -> p (c s)"), gs_ps[:])
            # ve = var + eps ; rstd = rsqrt(ve) with 2 Newton steps
            ve = small.tile([2, CCH], F, tag="gn_ve")
            nc.vector.tensor_mul(ve[:], gs[:, :, 0], gs[:, :, 0])
            nc.vector.tensor_sub(ve[:], gs[:, :, 1], ve[:])
            nc.vector.tensor_scalar_add(ve[:], ve[:], EPS)
            sd = small.tile([2, CCH], F, tag="gn_sd")
            nc.scalar.sqrt(sd[:], ve[:])
            r0 = small.tile([2, CCH], F, tag="gn_r0")
            nc.vector.reciprocal(r0[:], sd[:])
            for _ in range(2):
                t0 = small.tile([2, CCH], F, tag="gn_t0")
                nc.vector.tensor_mul(t0[:], r0[:], r0[:])
                nc.vector.tensor_mul(t0[:], t0[:], ve[:])
                nc.vector.tensor_scalar(
                    out=t0[:], in0=t0[:], scalar1=-0.5, scalar2=1.5,
                    op0=OP.mult, op1=OP.add)
                nc.vector.tensor_mul(r0[:], r0[:], t0[:])
            # broadcast (rstd, mean) back to all channels in one matmul
            rs = small.tile([2, CCH, 2], R, tag="gn_rs")
            nc.vector.tensor_copy(rs[:, :, 0], r0[:])
            nc.vector.tensor_copy(rs[:, :, 1], gs[:, :, 0])
            rm_ps = ps_gn.tile([128, 2 * CCH], F, tag="gn_ps")
            nc.tensor.matmul(rm_ps[:], bmask[:],
                             rs[:].rearrange("p c s -> p (c s)"),
                             start=True, stop=True)
            rm = small.tile([128, CCH, 2], F, tag="gn_rm")
            nc.vector.tensor_copy(rm[:].rearrange("p c s -> p (c s)"), rm_ps[:])
            # per-channel affine: xn = x*sc + oc
            sc = small.tile([128, CCH], F, tag="gn_sc")
            nc.vector.tensor_mul(sc[:], rm[:, :, 0], gamma_sb[:])
            oc = small.tile([128, CCH], F, tag="gn_oc")
            nc.vector.tensor_mul(oc[:], rm[:, :, 1], sc[:])
            nc.vector.tensor_sub(oc[:], beta_sb[:], oc[:])
            for c in range(CCH):
                nc.vector.tensor_scalar(
                    out=xn_sb[:, c, :], in0=x_sb[:, c, :],
                    scalar1=sc[:, c:c + 1], scalar2=oc[:, c:c + 1],
                    op0=OP.mult, op1=OP.add)
            for c in range(CCH):
                # fold output bias into the residual: x <- x + b_out
                nc.vector.tensor_scalar_add(
                    x_sb[:, c, :], x_sb[:, c, :], bout_sb[:, c:c + 1])

        # ================= QKV (+ pair-0 scores interleaved) ============
        e_tiles = {}
        av_tiles = {}

        def scores_pair_tt(p, tt):
            # 4 matmuls for head pair (2p, 2p+1), row groups 0/64 concurrent
            if tt == 0:
                for par in (0, 1):
                    e_sb = epool.tile([128, TT, S], BF, tag="e",
                                      name=f"e_{2 * p + par}")
                    e_tiles[2 * p + par] = e_sb
            pss = []
            for par in (0, 1):
                pst = ps_sc_pool.tile([128, S], F, tag="sc",
                                      name=f"sc_{p}_{tt}_{par}")
                pss.append(pst)
            for sh in range(SH):
                for par in (0, 1):
                    psl = slice(64 * par, 64 * par + 64)
                    nc.tensor.matmul(
                        pss[par][:, sh * NS:(sh + 1) * NS],
                        qk_sb[psl, 4 + p, tt * 128:(tt + 1) * 128],
                        qk_sb[psl, p, sh * NS:(sh + 1) * NS],
                        start=True, stop=True, tile_position=(64 * par, 0))
            for par in (0, 1):
                nc.scalar.activation(out=e_tiles[2 * p + par][:, tt, :],
                                     in_=pss[par][:], func=AF.Exp)

        with tc.tile_pool(name="ps_mm", bufs=4, space="PSUM") as ps_mm:
            def qk_chunk(j):  # output chunk j: q for 0-3, k for 4-7
                pss = []
                for sh in range(SH):
                    pst = ps_mm.tile([128, NS], F, tag="mm", name=f"mm_{j}_{sh}")
                    pss.append(pst)
                for c in range(CCH):
                    for sh in range(SH):  # stationary reused across halves
                        nc.tensor.matmul(
                            pss[sh][:], wq_sb[:, c, j * 128:(j + 1) * 128],
                            xn_sb[:, c, sh * NS:(sh + 1) * NS],
                            start=(c == 0), stop=(c == CCH - 1))
                for sh in range(SH):
                    nc.vector.tensor_copy(
                        qk_sb[:, j, sh * NS:(sh + 1) * NS], pss[sh][:])

            def v_chunk(t):  # vT chunk t: [128 seq, 512 channels]
                ps = ps_mm.tile([128, NS], F, tag="mm", name=f"mmv_{t}")
                for c in range(CCH):
                    nc.tensor.matmul(
                        ps[:], xn_sb[:, c, t * 128:(t + 1) * 128],
                        wq_sb[:, c, 2 * C:3 * C],
                        start=(c == 0), stop=(c == CCH - 1))
                nc.vector.tensor_copy(
                    vT_sb[:, t, :, 0:D],
                    ps[:].rearrange("p (h d) -> p h d", h=H))

            qk_chunk(0)
            qk_chunk(4)
            for t in range(TT):
                scores_pair_tt(0, t)
                v_chunk(t)
            for j in (1, 5, 2, 6, 3, 7):
                qk_chunk(j)

        # output-projection weights only needed at the tail
        nc.sync.dma_start(out=wo_sb, in_=woutT_d.rearrange("(c p) o -> p c o", p=128))

        # =========================== Attention ==========================
        att_sb = const.tile([128, CCH, S], R, tag="xn_att", name="att_sb")

        with tc.tile_pool(name="ps_av", bufs=2, space="PSUM") as ps_av_pool:

            def av2(h, t):
                p = h // 2
                if t == 0:
                    if p == H // 2 - 1:
                        # last pair: scores pool is idle by now
                        pst = ps_sc_pool.tile([D + 1, S], F, tag="sc",
                                              name=f"av_{h}")
                    else:
                        pst = ps_av_pool.tile([D + 1, S], F, tag="av",
                                              name=f"av_{h}")
                    av_tiles[h] = pst
                ps_av = av_tiles[h]
                e_sb = e_tiles[h]
                for sh in range(SH):
                    nc.tensor.matmul(
                        ps_av[:, sh * NS:(sh + 1) * NS],
                        vT_sb[:, t, h, :], e_sb[:, t, sh * NS:(sh + 1) * NS],
                        start=(t == 0), stop=(t == TT - 1))

            fin_state = {}

            def fin_pre(h):
                ps_av = av_tiles[h]
                e_tiles.pop(h, None)
                # denominator row -> [64, S/64] -> reciprocal -> row
                den_row = rpool.tile([1, S], F, tag="denrow", name=f"dr_{h}")
                nc.vector.tensor_copy(den_row[:], ps_av[D:D + 1, :])
                denT = rpool.tile([64, S // 64], F, tag="denT", name=f"dt_{h}")
                nc.sync.dma_start(out=denT, in_=den_row)
                rdenT = rpool.tile([64, S // 64], R, tag="rdenT", name=f"rt_{h}")
                with nc.allow_low_precision(reason="float32r is bitwise fp32"):
                    nc.vector.reciprocal(rdenT[:], denT[:])
                rden_d = dpool.tile([1, S], R, tag="rdend", name=f"rdd_{h}")
                nc.sync.dma_start(out=rden_d, in_=rdenT)
                # replicate the reciprocal row to 64 partitions (DRAM
                # APs allow a zero-step partition broadcast)
                rb_sb = rpool.tile([64, S], R, tag="rb", name=f"rb_{h}")
                import concourse.bass as bass_mod
                rden_bcast = bass_mod.AP(
                    tensor=rden_d.tensor, offset=rden_d[:].offset,
                    ap=[[0, 64]] + rden_d[:].ap[1:])
                nc.sync.dma_start(out=rb_sb, in_=rden_bcast)
                fin_state[h] = rb_sb

            def fin_post(h):
                hc, p0 = h // 2, 64 * (h % 2)
                ps_av = av_tiles.pop(h)
                rb_sb = fin_state.pop(h)
                dst = att_sb[p0:p0 + 64, hc, :]
                nc.vector.tensor_mul(dst, ps_av[0:D, :], rb_sb[:])

            # Per pair-iteration: head 2p drains in the first half at double
            # rate, head 2p+1 in the second half, so each fin chain overlaps
            # the following scores instead of blocking the PE stream.
            pending_post = None
            for p in range(H // 2):
                for tt in range(TT):
                    if p + 1 < H // 2:
                        scores_pair_tt(p + 1, tt)
                    if pending_post is not None and tt == 2:
                        fin_post(pending_post)
                        pending_post = None
                    if tt < TT // 2:
                        av2(2 * p, 2 * tt)
                        av2(2 * p, 2 * tt + 1)
                    else:
                        av2(2 * p + 1, 2 * (tt - TT // 2))
                        av2(2 * p + 1, 2 * (tt - TT // 2) + 1)
                    if tt == 4:
                        fin_pre(2 * p)
                    if tt == 6:
                        fin_post(2 * p)
                fin_pre(2 * p + 1)
                pending_post = 2 * p + 1

            # tail: partial output projection (c<3) overlaps the last fin
            op_tiles = []
            for j in range(CCH):
                if j < 2:
                    pst = ps_av_pool.tile([128, S], F, tag="av", name=f"op_{j}")
                else:
                    pst = ps_sc_pool.tile([128, S], F, tag="sc", name=f"op_{j}")
                op_tiles.append(pst)
                for c in range(CCH - 1):
                    for sh in range(SH):
                        nc.tensor.matmul(
                            pst[:, sh * NS:(sh + 1) * NS],
                            wo_sb[:, c, j * 128:(j + 1) * 128],
                            att_sb[:, c, sh * NS:(sh + 1) * NS],
                            start=(c == 0), stop=False)
                if j == 1 and pending_post is not None:
                    fin_post(pending_post)
                    pending_post = None
            for j in range(CCH):
                pst = op_tiles[j]
                c = CCH - 1
                for sh in range(SH):
                    nc.tensor.matmul(
                        pst[:, sh * NS:(sh + 1) * NS],
                        wo_sb[:, c, j * 128:(j + 1) * 128],
                        att_sb[:, c, sh * NS:(sh + 1) * NS],
                        start=False, stop=True)
                for sh in range(SH):
                    ot = evac.tile([128, NS], R, tag="ot", name=f"ot_{j}_{sh}")
                    nc.vector.tensor_add(
                        ot[:], pst[:, sh * NS:(sh + 1) * NS],
                        x_sb[:, j, sh * NS:(sh + 1) * NS])
                    nc.sync.dma_start(
                        out=out_d.rearrange("(c p) s -> p c s", p=128)
                            [:, j, sh * NS:(sh + 1) * NS],
                        in_=ot[:])



def kernel(x, gamma, beta, w_qkv, w_out, b_out):
    from concourse.bass_utils import run_bass_kernel_spmd

    if "nc" not in _CACHE:
        _CACHE["nc"] = _build()
    nc = _CACHE["nc"]

    x = np.ascontiguousarray(x, dtype=np.float32)
    # host-side layout prep: transpose weights for [K=channel] matmuls and
    # fold the 1/sqrt(D) score scale into w_q
    wqkvT = np.ascontiguousarray(np.asarray(w_qkv).T, dtype=np.float32).copy()
    wqkvT[:, 0:C] *= np.float32(1.0 / np.sqrt(D))
    woutT = np.ascontiguousarray(np.asarray(w_out).T, dtype=np.float32)
    gamma = np.ascontiguousarray(gamma, dtype=np.float32)
    beta = np.ascontiguousarray(beta, dtype=np.float32)
    b_out = np.ascontiguousarray(b_out, dtype=np.float32)

    gmask = np.zeros((128, 2), dtype=np.float32)
    gmask[0:64, 0] = 1.0 / 64
    gmask[64:128, 1] = 1.0 / 64
    bmask = np.zeros((2, 128), dtype=np.float32)
    bmask[0, 0:64] = 1.0
    bmask[1, 64:128] = 1.0
    import ml_dtypes
    onesr = np.ones((1, 64), dtype=np.float32)
    ident = np.eye(128, dtype=np.float32)
    onesv = np.ones((128, 64), dtype=ml_dtypes.bfloat16)
    in_maps = [
        {"x": x[b], "wqkvT": wqkvT, "woutT": woutT,
         "gamma": gamma, "beta": beta, "bout": b_out,
         "gmask": gmask, "bmask": bmask, "onesr": onesr, "onesv": onesv,
         "ident": ident}
        for b in range(B)
    ]
    res = run_bass_kernel_spmd(nc, in_maps, core_ids=list(range(B)), trace=False)
    return np.stack([res.results[b]["out"] for b in range(B)], axis=0)


# revision 20
# speedup vs baseline: 1.1460x; 1.1460x over previous
"""AttentionBlock1D Trainium2 kernel.

Reference computation (per batch element b):
    xn   = GroupNorm(x, 8 groups, eps=1e-5) * gamma + beta
    qkv  = w_qkv @ xn                  # [3C, S], 1x1 conv == channel matmul
    q,k,v split; heads H=8, D=64
    att  = softmax(q^T k / sqrt(D)) v  # per head
    out  = w_out @ att + b_out + x

Distribution: pure data parallelism — batch B=8, one batch element per
NeuronCore (8 cores). Weights replicated.

Per-core dataflow (matmuls in float32r [tf32-like, full PE rate at
N>=256]; the AV stage runs bf16; fp32 PSUM accumulation everywhere):
    - GroupNorm stats via bn_stats/bn_aggr per channel; group reduce and
      broadcast-back are tiny masked matmuls (cross-partition on PE).
    - QKV keeps q,k in [channel, seq] layout; v is produced transposed
      ([seq, channel]) by swapping matmul operands, with a ones column
      appended per head (softmax denominator trick).
    - scores^T[t,s] = (k_h)^T q_h computed directly in [t, s] layout so
      softmax's exp reads PSUM and the AV matmul needs no transposes.
      Max-subtraction is skipped (scores are O(5); exp safe in fp32),
      mathematically identical to jax.nn.softmax.
    - exp on ScalarE (critical engine: S*S*H = 8.4M activations),
      emitted two heads ahead of the AV consumer.
    - AV accumulates [d, s] plus a denominator row over t chunks; the
      normalization multiplies by a K=1 broadcast matmul of the
      reciprocal denominator during PSUM evacuation.
    - output projection accumulates over channel chunks and adds
      (x + b_out) during evacuation.
"""

import numpy as np

B, C, S = 8, 512, 1024
H = 8            # heads
D = C // H       # 64 head dim
EPS = 1e-5
CCH = C // 128   # 4 channel chunks of 128
TT = S // 128    # 8 t (key) chunks of 128
NS = 512         # matmul moving free dim
SH = S // NS     # 2 s halves

_CACHE = {}


def _build(e_bufs=3):
    import concourse.bacc as bacc
    import concourse.tile as tile
    import concourse.mybir as mybir

    nc = bacc.Bacc("TRN2", target_bir_lowering=False, debug=False)
    R = mybir.dt.float32r
    F = mybir.dt.float32

    x_d = nc.dram_tensor("x", [C, S], R, kind="ExternalInput").ap()
    wqkvT_d = nc.dram_tensor("wqkvT", [C, 3 * C], R, kind="ExternalInput").ap()
    woutT_d = nc.dram_tensor("woutT", [C, C], R, kind="ExternalInput").ap()
    gamma_d = nc.dram_tensor("gamma", [C], F, kind="ExternalInput").ap()
    beta_d = nc.dram_tensor("beta", [C], F, kind="ExternalInput").ap()
    bout_d = nc.dram_tensor("bout", [C], F, kind="ExternalInput").ap()
    gmask_d = nc.dram_tensor("gmask", [128, 2], R, kind="ExternalInput").ap()
    bmask_d = nc.dram_tensor("bmask", [2, 128], R, kind="ExternalInput").ap()
    onesr_d = nc.dram_tensor("onesr", [1, 64], R, kind="ExternalInput").ap()
    ident_d = nc.dram_tensor("ident", [128, 128], R, kind="ExternalInput").ap()
    onesv_d = nc.dram_tensor("onesv", [128, 64], mybir.dt.bfloat16,
                             kind="ExternalInput").ap()
    out_d = nc.dram_tensor("out", [C, S], R, kind="ExternalOutput").ap()

    with tile.TileContext(nc) as tc:
        _body(tc, nc, mybir, x_d, wqkvT_d, woutT_d, gamma_d, beta_d, bout_d,
              gmask_d, bmask_d, onesr_d, onesv_d, ident_d, out_d, e_bufs)
    nc.compile()
    return nc


def _body(tc, nc, mybir, x_d, wqkvT_d, woutT_d, gamma_d, beta_d, bout_d,
          gmask_d, bmask_d, onesr_d, onesv_d, ident_d, out_d, e_bufs):
    from contextlib import ExitStack

    R = mybir.dt.float32r
    F = mybir.dt.float32
    BF = mybir.dt.bfloat16
    OP = mybir.AluOpType
    AF = mybir.ActivationFunctionType

    ctx = ExitStack()
    with ctx:
        const = ctx.enter_context(tc.tile_pool(name="const", bufs=1))
        small = ctx.enter_context(tc.tile_pool(name="small", bufs=4))
        rpool = ctx.enter_context(tc.tile_pool(name="rpool", bufs=2))
        dpool = ctx.enter_context(tc.tile_pool(name="dpool", bufs=2, space="DRAM"))
        epool = ctx.enter_context(tc.tile_pool(name="epool", bufs=4))
        evac = ctx.enter_context(tc.tile_pool(name="evac", bufs=2))

        # ---- resident tensors ----
        x_sb = const.tile([128, CCH, S], R, tag="x")        # x, later x + b_out
        wq_sb = const.tile([128, CCH, 3 * C], R, tag="wq")
        wo_sb = const.tile([128, CCH, C], R, tag="wo")
        gamma_sb = const.tile([128, CCH], F, tag="gam")
        beta_sb = const.tile([128, CCH], F, tag="bet")
        bout_sb = const.tile([128, CCH], F, tag="bou")
        qk_sb = const.tile([128, 2 * CCH, S], R, tag="qk")  # q: 0-3, k: 4-7
        vT_sb = const.tile([128, TT, H, D + 1], BF, tag="vt")  # +ones column
        # xn shares space with att (xn dead after QKV, att written after)
        xn_sb = const.tile([128, CCH, S], R, tag="xn_att")
        att_sb = None  # allocated later from the same tag

        for c in range(CCH):
            nc.sync.dma_start(out=x_sb[:, c, :],
                              in_=x_d.rearrange("(c p) s -> p c s", p=128)[:, c, :])
        nc.sync.dma_start(out=gamma_sb, in_=gamma_d.rearrange("(c p) -> p c", p=128))
        nc.sync.dma_start(out=beta_sb, in_=beta_d.rearrange("(c p) -> p c", p=128))
        nc.sync.dma_start(out=bout_sb, in_=bout_d.rearrange("(c p) -> p c", p=128))
        for c in range(CCH):
            nc.sync.dma_start(out=wq_sb[:, c, :],
                              in_=wqkvT_d.rearrange("(c p) o -> p c o", p=128)[:, c, :])

        gmask = const.tile([128, 2], R, tag="gmask")   # [channel, group] 1/64
        bmask = const.tile([2, 128], R, tag="bmask")   # [group, channel] 1
        ones_row = const.tile([1, 64], R, tag="ones")  # K=1 broadcast stationary
        ident_sb = const.tile([128, 128], R, tag="ident")
        nc.sync.dma_start(out=gmask, in_=gmask_d)
        nc.sync.dma_start(out=bmask, in_=bmask_d)
        nc.sync.dma_start(out=ones_row, in_=onesr_d)
        nc.sync.dma_start(out=ident_sb, in_=ident_d)
        nc.sync.dma_start(
            out=vT_sb[:, :, :, D:D + 1],
            in_=onesv_d.rearrange("p (t h) -> p t h", t=TT)[:, :, :, None])

        # scores/exp psum pool lives from the start (banks 0-3, LIFO inside)
        ps_sc_pool = ctx.enter_context(
            tc.tile_pool(name="ps_sc", bufs=2, space="PSUM"))

        # =========================== GroupNorm ===========================
        # Stage-batched across the 4 channel chunks to minimize
        # cross-engine ping-pong: one group-reduce matmul and one
        # broadcast-back matmul handle all 8 groups at once.
        with tc.tile_pool(name="ps_gn", bufs=2, space="PSUM") as ps_gn:
            st = small.tile([128, CCH, 2, 6], F, tag="gn_st")
            for c in range(CCH):
                nc.vector.bn_stats(st[:, c, 0, :], x_sb[:, c, 0:512])
                nc.vector.bn_stats(st[:, c, 1, :], x_sb[:, c, 512:1024])
            mv = small.tile([128, CCH, 2], F, tag="gn_mv")  # per-ch mean, var
            for c in range(CCH):
                nc.vector.bn_aggr(mv[:, c, :], st[:, c, :, :])
            ms = small.tile([128, CCH, 2], R, tag="gn_ms")  # mean, E[x^2]
            nc.vector.tensor_copy(ms[:, :, 0:1], mv[:, :, 0:1])
            nc.vector.tensor_mul(ms[:, :, 1:2], mv[:, :, 0:1], mv[:, :, 0:1])
            nc.vector.tensor_add(ms[:, :, 1:2], ms[:, :, 1:2], mv[:, :, 1:2])
            # one group reduce for all chunks: [2, (chunk, stat)]
            gs_ps = ps_gn.tile([2, 2 * CCH], F, tag="gn_ps")
            nc.tensor.matmul(gs_ps[:], gmask[:],
                             ms[:].rearrange("p c s -> p (c s)"),
                             start=True, stop=True)
            gs = small.tile([2, CCH, 2], F, tag="gn_gs")
            nc.vector.tensor_copy(gs[:].rearrange("p c s -> p (c s)"), gs_ps[:])
            # ve = var + eps ; rstd = rsqrt(ve) with 2 Newton steps
            ve = small.tile([2, CCH], F, tag="gn_ve")
            nc.vector.tensor_mul(ve[:], gs[:, :, 0], gs[:, :, 0])
            nc.vector.tensor_sub(ve[:], gs[:, :, 1], ve[:])
            nc.vector.tensor_scalar_add(ve[:], ve[:], EPS)
            sd = small.tile([2, CCH], F, tag="gn_sd")
            nc.scalar.sqrt(sd[:], ve[:])
            r0 = small.tile([2, CCH], F, tag="gn_r0")
            nc.vector.reciprocal(r0[:], sd[:])
            for _ in range(2):
                t0 = small.tile([2, CCH], F, tag="gn_t0")
                nc.vector.tensor_mul(t0[:], r0[:], r0[:])
                nc.vector.tensor_mul(t0[:], t0[:], ve[:])
                nc.vector.tensor_scalar(
                    out=t0[:], in0=t0[:], scalar1=-0.5, scalar2=1.5,
                    op0=OP.mult, op1=OP.add)
                nc.vector.tensor_mul(r0[:], r0[:], t0[:])
            # broadcast (rstd, mean) back to all channels in one matmul
            rs = small.tile([2, CCH, 2], R, tag="gn_rs")
            nc.vector.tensor_copy(rs[:, :, 0], r0[:])
            nc.vector.tensor_copy(rs[:, :, 1], gs[:, :, 0])
            rm_ps = ps_gn.tile([128, 2 * CCH], F, tag="gn_ps")
            nc.tensor.matmul(rm_ps[:], bmask[:],
                             rs[:].rearrange("p c s -> p (c s)"),
                             start=True, stop=True)
            rm = small.tile([128, CCH, 2], F, tag="gn_rm")
            nc.vector.tensor_copy(rm[:].rearrange("p c s -> p (c s)"), rm_ps[:])
            # per-channel affine: xn = x*sc + oc
            sc = small.tile([128, CCH], F, tag="gn_sc")
            nc.vector.tensor_mul(sc[:], rm[:, :, 0], gamma_sb[:])
            oc = small.tile([128, CCH], F, tag="gn_oc")
            nc.vector.tensor_mul(oc[:], rm[:, :, 1], sc[:])
            nc.vector.tensor_sub(oc[:], beta_sb[:], oc[:])
            for c in range(CCH):
                nc.vector.tensor_scalar(
                    out=xn_sb[:, c, :], in0=x_sb[:, c, :],
                    scalar1=sc[:, c:c + 1], scalar2=oc[:, c:c + 1],
                    op0=OP.mult, op1=OP.add)
                # fold output bias into the residual: x <- x + b_out
                nc.vector.tensor_scalar_add(
                    x_sb[:, c, :], x_sb[:, c, :], bout_sb[:, c:c + 1])

        # ================= QKV (+ pair-0 scores interleaved) ============
        e_tiles = {}
        av_tiles = {}

        def scores_pair_tt(p, tt):
            # 4 matmuls for head pair (2p, 2p+1), row groups 0/64 concurrent
            if tt == 0:
                for par in (0, 1):
                    e_sb = epool.tile([128, TT, S], BF, tag="e",
                                      name=f"e_{2 * p + par}")
                    e_tiles[2 * p + par] = e_sb
            pss = []
            for par in (0, 1):
                pst = ps_sc_pool.tile([128, S], F, tag="sc",
                                      name=f"sc_{p}_{tt}_{par}")
                pss.append(pst)
            for sh in range(SH):
                for par in (0, 1):
                    psl = slice(64 * par, 64 * par + 64)
                    nc.tensor.matmul(
                        pss[par][:, sh * NS:(sh + 1) * NS],
                        qk_sb[psl, 4 + p, tt * 128:(tt + 1) * 128],
                        qk_sb[psl, p, sh * NS:(sh + 1) * NS],
                        start=True, stop=True, tile_position=(64 * par, 0))
            for par in (0, 1):
                nc.scalar.activation(out=e_tiles[2 * p + par][:, tt, :],
                                     in_=pss[par][:], func=AF.Exp)

        with tc.tile_pool(name="ps_mm", bufs=4, space="PSUM") as ps_mm:
            def qk_chunk(j):  # output chunk j: q for 0-3, k for 4-7
                pss = []
                for sh in range(SH):
                    pst = ps_mm.tile([128, NS], F, tag="mm", name=f"mm_{j}_{sh}")
                    pss.append(pst)
                for c in range(CCH):
                    for sh in range(SH):  # stationary reused across halves
                        nc.tensor.matmul(
                            pss[sh][:], wq_sb[:, c, j * 128:(j + 1) * 128],
                            xn_sb[:, c, sh * NS:(sh + 1) * NS],
                            start=(c == 0), stop=(c == CCH - 1))
                for sh in range(SH):
                    nc.vector.tensor_copy(
                        qk_sb[:, j, sh * NS:(sh + 1) * NS], pss[sh][:])

            def v_chunk(t):  # vT chunk t: [128 seq, 512 channels]
                ps = ps_mm.tile([128, NS], F, tag="mm", name=f"mmv_{t}")
                for c in range(CCH):
                    nc.tensor.matmul(
                        ps[:], xn_sb[:, c, t * 128:(t + 1) * 128],
                        wq_sb[:, c, 2 * C:3 * C],
                        start=(c == 0), stop=(c == CCH - 1))
                nc.vector.tensor_copy(
                    vT_sb[:, t, :, 0:D],
                    ps[:].rearrange("p (h d) -> p h d", h=H))

            qk_chunk(0)
            qk_chunk(4)
            for t in range(TT):
                scores_pair_tt(0, t)
                v_chunk(t)
            for j in (1, 5, 2, 6, 3, 7):
                qk_chunk(j)

        # output-projection weights only needed at the tail
        nc.sync.dma_start(out=wo_sb, in_=woutT_d.rearrange("(c p) o -> p c o", p=128))

        # =========================== Attention ==========================
        att_sb = const.tile([128, CCH, S], R, tag="xn_att", name="att_sb")

        with tc.tile_pool(name="ps_av", bufs=2, space="PSUM") as ps_av_pool:

            def av2(h, t):
                p = h // 2
                if t == 0:
                    if p == H // 2 - 1:
                        # last pair: scores pool is idle by now
                        pst = ps_sc_pool.tile([D + 1, S], F, tag="sc",
                                              name=f"av_{h}")
                    else:
                        pst = ps_av_pool.tile([D + 1, S], F, tag="av",
                                              name=f"av_{h}")
                    av_tiles[h] = pst
                ps_av = av_tiles[h]
                e_sb = e_tiles[h]
                for sh in range(SH):
                    nc.tensor.matmul(
                        ps_av[:, sh * NS:(sh + 1) * NS],
                        vT_sb[:, t, h, :], e_sb[:, t, sh * NS:(sh + 1) * NS],
                        start=(t == 0), stop=(t == TT - 1))

            fin_state = {}

            def fin_pre(h):
                ps_av = av_tiles[h]
                e_tiles.pop(h, None)
                # denominator row -> [64, S/64] -> reciprocal -> row
                den_row = rpool.tile([1, S], F, tag="denrow", name=f"dr_{h}")
                nc.vector.tensor_copy(den_row[:], ps_av[D:D + 1, :])
                denT = rpool.tile([64, S // 64], F, tag="denT", name=f"dt_{h}")
                nc.sync.dma_start(out=denT, in_=den_row)
                rdenT = rpool.tile([64, S // 64], R, tag="rdenT", name=f"rt_{h}")
                with nc.allow_low_precision(reason="float32r is bitwise fp32"):
                    nc.vector.reciprocal(rdenT[:], denT[:])
                rden_d = dpool.tile([1, S], R, tag="rdend", name=f"rdd_{h}")
                nc.sync.dma_start(out=rden_d, in_=rdenT)
                # replicate the reciprocal row to 64 partitions (DRAM
                # APs allow a zero-step partition broadcast)
                rb_sb = rpool.tile([64, S], R, tag="rb", name=f"rb_{h}")
                import concourse.bass as bass_mod
                rden_bcast = bass_mod.AP(
                    tensor=rden_d.tensor, offset=rden_d[:].offset,
                    ap=[[0, 64]] + rden_d[:].ap[1:])
                nc.sync.dma_start(out=rb_sb, in_=rden_bcast)
                fin_state[h] = rb_sb

            def fin_post(h):
                hc, p0 = h // 2, 64 * (h % 2)
                ps_av = av_tiles.pop(h)
                rb_sb = fin_state.pop(h)
                dst = att_sb[p0:p0 + 64, hc, :]
                nc.vector.tensor_mul(dst, ps_av[0:D, :], rb_sb[:])

            # Per pair-iteration: head 2p drains in the first half at double
            # rate, head 2p+1 in the second half, so each fin chain overlaps
            # the following scores instead of blocking the PE stream.
            pending_post = None
            for p in range(H // 2):
                for tt in range(TT):
                    if p + 1 < H // 2:
                        scores_pair_tt(p + 1, tt)
                    if pending_post is not None and tt == 2:
                        fin_post(pending_post)
                        pending_post = None
                    if tt < TT // 2:
                        av2(2 * p, 2 * tt)
                        av2(2 * p, 2 * tt + 1)
                    else:
                        av2(2 * p + 1, 2 * (tt - TT // 2))
                        av2(2 * p + 1, 2 * (tt - TT // 2) + 1)
                    if tt == 4:
                        fin_pre(2 * p)
                    if tt == 6:
                        fin_post(2 * p)
                fin_pre(2 * p + 1)
                pending_post = 2 * p + 1

            # tail: partial output projection (c<3) overlaps the last fin
            op_tiles = []
            for j in range(CCH):
                if j < 2:
                    pst = ps_av_pool.tile([128, S], F, tag="av", name=f"op_{j}")
                else:
                    pst = ps_sc_pool.tile([128, S], F, tag="sc", name=f"op_{j}")
                op_tiles.append(pst)
                for c in range(CCH - 1):
                    for sh in range(SH):
                        nc.tensor.matmul(
                            pst[:, sh * NS:(sh + 1) * NS],
                            wo_sb[:, c, j * 128:(j + 1) * 128],
                            att_sb[:, c, sh * NS:(sh + 1) * NS],
                            start=(c == 0), stop=False)
                if j == 1 and pending_post is not None:
                    fin_post(pending_post)
                    pending_post = None
            for j in range(CCH):
                pst = op_tiles[j]
                c = CCH - 1
                for sh in range(SH):
                    nc.tensor.matmul(
                        pst[:, sh * NS:(sh + 1) * NS],
                        wo_sb[:, c, j * 128:(j + 1) * 128],
                        att_sb[:, c, sh * NS:(sh + 1) * NS],
                        start=False, stop=True)
                for sh in range(SH):
                    ot = evac.tile([128, NS], R, tag="ot", name=f"ot_{j}_{sh}")
                    nc.vector.tensor_add(
                        ot[:], pst[:, sh * NS:(sh + 1) * NS],
                        x_sb[:, j, sh * NS:(sh + 1) * NS])
                    nc.sync.dma_start(
                        out=out_d.rearrange("(c p) s -> p c s", p=128)
                            [:, j, sh * NS:(sh + 1) * NS],
                        in_=ot[:])



def kernel(x, gamma, beta, w_qkv, w_out, b_out):
    from concourse.bass_utils import run_bass_kernel_spmd

    if "nc" not in _CACHE:
        _CACHE["nc"] = _build()
    nc = _CACHE["nc"]

    x = np.ascontiguousarray(x, dtype=np.float32)
    # host-side layout prep: transpose weights for [K=channel] matmuls and
    # fold the 1/sqrt(D) score scale into w_q
    wqkvT = np.ascontiguousarray(np.asarray(w_qkv).T, dtype=np.float32).copy()
    wqkvT[:, 0:C] *= np.float32(1.0 / np.sqrt(D))
    woutT = np.ascontiguousarray(np.asarray(w_out).T, dtype=np.float32)
    gamma = np.ascontiguousarray(gamma, dtype=np.float32)
    beta = np.ascontiguousarray(beta, dtype=np.float32)
    b_out = np.ascontiguousarray(b_out, dtype=np.float32)

    gmask = np.zeros((128, 2), dtype=np.float32)
    gmask[0:64, 0] = 1.0 / 64
    gmask[64:128, 1] = 1.0 / 64
    bmask = np.zeros((2, 128), dtype=np.float32)
    bmask[0, 0:64] = 1.0
    bmask[1, 64:128] = 1.0
    import ml_dtypes
    onesr = np.ones((1, 64), dtype=np.float32)
    ident = np.eye(128, dtype=np.float32)
    onesv = np.ones((128, 64), dtype=ml_dtypes.bfloat16)
    in_maps = [
        {"x": x[b], "wqkvT": wqkvT, "woutT": woutT,
         "gamma": gamma, "beta": beta, "bout": b_out,
         "gmask": gmask, "bmask": bmask, "onesr": onesr, "onesv": onesv,
         "ident": ident}
        for b in range(B)
    ]
    res = run_bass_kernel_spmd(nc, in_maps, core_ids=list(range(B)), trace=False)
    return np.stack([res.results[b]["out"] for b in range(B)], axis=0)


# revision 21
# speedup vs baseline: 1.1653x; 1.0168x over previous
"""AttentionBlock1D Trainium2 kernel.

Reference computation (per batch element b):
    xn   = GroupNorm(x, 8 groups, eps=1e-5) * gamma + beta
    qkv  = w_qkv @ xn                  # [3C, S], 1x1 conv == channel matmul
    q,k,v split; heads H=8, D=64
    att  = softmax(q^T k / sqrt(D)) v  # per head
    out  = w_out @ att + b_out + x

Distribution: pure data parallelism — batch B=8, one batch element per
NeuronCore (8 cores). Weights replicated.

Per-core dataflow (matmuls in float32r [tf32-like, full PE rate at
N>=256]; the AV stage runs bf16; fp32 PSUM accumulation everywhere):
    - GroupNorm stats via bn_stats/bn_aggr per channel; group reduce and
      broadcast-back are tiny masked matmuls (cross-partition on PE).
    - QKV keeps q,k in [channel, seq] layout; v is produced transposed
      ([seq, channel]) by swapping matmul operands, with a ones column
      appended per head (softmax denominator trick).
    - scores^T[t,s] = (k_h)^T q_h computed directly in [t, s] layout so
      softmax's exp reads PSUM and the AV matmul needs no transposes.
      Max-subtraction is skipped (scores are O(5); exp safe in fp32),
      mathematically identical to jax.nn.softmax.
    - exp on ScalarE (critical engine: S*S*H = 8.4M activations),
      emitted two heads ahead of the AV consumer.
    - AV accumulates [d, s] plus a denominator row over t chunks; the
      normalization multiplies by a K=1 broadcast matmul of the
      reciprocal denominator during PSUM evacuation.
    - output projection accumulates over channel chunks and adds
      (x + b_out) during evacuation.
"""

import numpy as np

B, C, S = 8, 512, 1024
H = 8            # heads
D = C // H       # 64 head dim
EPS = 1e-5
CCH = C // 128   # 4 channel chunks of 128
TT = S // 128    # 8 t (key) chunks of 128
NS = 512         # matmul moving free dim
SH = S // NS     # 2 s halves

_CACHE = {}


def _build(e_bufs=3):
    import concourse.bacc as bacc
    import concourse.tile as tile
    import concourse.mybir as mybir

    nc = bacc.Bacc("TRN2", target_bir_lowering=False, debug=False)
    R = mybir.dt.float32r
    F = mybir.dt.float32

    x_d = nc.dram_tensor("x", [C, S], R, kind="ExternalInput").ap()
    wqkvT_d = nc.dram_tensor("wqkvT", [C, 3 * C], R, kind="ExternalInput").ap()
    woutT_d = nc.dram_tensor("woutT", [C, C], R, kind="ExternalInput").ap()
    gamma_d = nc.dram_tensor("gamma", [C], F, kind="ExternalInput").ap()
    beta_d = nc.dram_tensor("beta", [C], F, kind="ExternalInput").ap()
    bout_d = nc.dram_tensor("bout", [C], F, kind="ExternalInput").ap()
    gmask_d = nc.dram_tensor("gmask", [128, 2], R, kind="ExternalInput").ap()
    bmask_d = nc.dram_tensor("bmask", [2, 128], R, kind="ExternalInput").ap()
    onesr_d = nc.dram_tensor("onesr", [1, 64], R, kind="ExternalInput").ap()
    ident_d = nc.dram_tensor("ident", [128, 128], R, kind="ExternalInput").ap()
    onesv_d = nc.dram_tensor("onesv", [128, 64], mybir.dt.bfloat16,
                             kind="ExternalInput").ap()
    out_d = nc.dram_tensor("out", [C, S], R, kind="ExternalOutput").ap()

    with tile.TileContext(nc) as tc:
        _body(tc, nc, mybir, x_d, wqkvT_d, woutT_d, gamma_d, beta_d, bout_d,
              gmask_d, bmask_d, onesr_d, onesv_d, ident_d, out_d, e_bufs)
    nc.compile()
    return nc


def _body(tc, nc, mybir, x_d, wqkvT_d, woutT_d, gamma_d, beta_d, bout_d,
          gmask_d, bmask_d, onesr_d, onesv_d, ident_d, out_d, e_bufs):
    from contextlib import ExitStack

    R = mybir.dt.float32r
    F = mybir.dt.float32
    BF = mybir.dt.bfloat16
    OP = mybir.AluOpType
    AF = mybir.ActivationFunctionType

    ctx = ExitStack()
    with ctx:
        const = ctx.enter_context(tc.tile_pool(name="const", bufs=1))
        small = ctx.enter_context(tc.tile_pool(name="small", bufs=4))
        rpool = ctx.enter_context(tc.tile_pool(name="rpool", bufs=2))
        dpool = ctx.enter_context(tc.tile_pool(name="dpool", bufs=2, space="DRAM"))
        epool = ctx.enter_context(tc.tile_pool(name="epool", bufs=4))
        evac = ctx.enter_context(tc.tile_pool(name="evac", bufs=2))

        # ---- resident tensors ----
        x_sb = const.tile([128, CCH, S], R, tag="x")        # x, later x + b_out
        wq_sb = const.tile([128, CCH, 3 * C], R, tag="wq")
        wo_sb = const.tile([128, CCH, C], R, tag="wo")
        gamma_sb = const.tile([128, CCH], F, tag="gam")
        beta_sb = const.tile([128, CCH], F, tag="bet")
        bout_sb = const.tile([128, CCH], F, tag="bou")
        qk_sb = const.tile([128, 2 * CCH, S], R, tag="qk")  # q: 0-3, k: 4-7
        vT_sb = const.tile([128, TT, H, D + 1], BF, tag="vt")  # +ones column
        # xn shares space with att (xn dead after QKV, att written after)
        xn_sb = const.tile([128, CCH, S], R, tag="xn_att")
        att_sb = None  # allocated later from the same tag

        for c in range(CCH):
            nc.sync.dma_start(out=x_sb[:, c, :],
                              in_=x_d.rearrange("(c p) s -> p c s", p=128)[:, c, :])
        nc.sync.dma_start(out=gamma_sb, in_=gamma_d.rearrange("(c p) -> p c", p=128))
        nc.sync.dma_start(out=beta_sb, in_=beta_d.rearrange("(c p) -> p c", p=128))
        nc.sync.dma_start(out=bout_sb, in_=bout_d.rearrange("(c p) -> p c", p=128))
        for c in range(CCH):
            nc.sync.dma_start(out=wq_sb[:, c, :],
                              in_=wqkvT_d.rearrange("(c p) o -> p c o", p=128)[:, c, :])

        gmask = const.tile([128, 2], R, tag="gmask")   # [channel, group] 1/64
        bmask = const.tile([2, 128], R, tag="bmask")   # [group, channel] 1
        ones_row = const.tile([1, 64], R, tag="ones")  # K=1 broadcast stationary
        ident_sb = const.tile([128, 128], R, tag="ident")
        nc.sync.dma_start(out=gmask, in_=gmask_d)
        nc.sync.dma_start(out=bmask, in_=bmask_d)
        nc.sync.dma_start(out=ones_row, in_=onesr_d)
        nc.sync.dma_start(out=ident_sb, in_=ident_d)
        nc.sync.dma_start(
            out=vT_sb[:, :, :, D:D + 1],
            in_=onesv_d.rearrange("p (t h) -> p t h", t=TT)[:, :, :, None])

        # scores/exp psum pool lives from the start (banks 0-3, LIFO inside)
        ps_sc_pool = ctx.enter_context(
            tc.tile_pool(name="ps_sc", bufs=2, space="PSUM"))

        # =========================== GroupNorm ===========================
        # Stage-batched across the 4 channel chunks to minimize
        # cross-engine ping-pong: one group-reduce matmul and one
        # broadcast-back matmul handle all 8 groups at once.
        with tc.tile_pool(name="ps_gn", bufs=2, space="PSUM") as ps_gn:
            st = small.tile([128, CCH, 2, 6], F, tag="gn_st")
            for c in range(CCH):
                nc.vector.bn_stats(st[:, c, 0, :], x_sb[:, c, 0:512])
                nc.vector.bn_stats(st[:, c, 1, :], x_sb[:, c, 512:1024])
            mv = small.tile([128, CCH, 2], F, tag="gn_mv")  # per-ch mean, var
            for c in range(CCH):
                nc.vector.bn_aggr(mv[:, c, :], st[:, c, :, :])
            ms = small.tile([128, CCH, 2], R, tag="gn_ms")  # mean, E[x^2]
            nc.vector.tensor_copy(ms[:, :, 0:1], mv[:, :, 0:1])
            nc.vector.tensor_mul(ms[:, :, 1:2], mv[:, :, 0:1], mv[:, :, 0:1])
            nc.vector.tensor_add(ms[:, :, 1:2], ms[:, :, 1:2], mv[:, :, 1:2])
            # one group reduce for all chunks: [2, (chunk, stat)]
            gs_ps = ps_gn.tile([2, 2 * CCH], F, tag="gn_ps")
            nc.tensor.matmul(gs_ps[:], gmask[:],
                             ms[:].rearrange("p c s -> p (c s)"),
                             start=True, stop=True)
            gs = small.tile([2, CCH, 2], F, tag="gn_gs")
            nc.vector.tensor_copy(gs[:].rearrange("p c s -> p (c s)"), gs_ps[:])
            # ve = var + eps ; rstd = rsqrt(ve) with 2 Newton steps
            ve = small.tile([2, CCH], F, tag="gn_ve")
            nc.vector.tensor_mul(ve[:], gs[:, :, 0], gs[:, :, 0])
            nc.vector.tensor_sub(ve[:], gs[:, :, 1], ve[:])
            nc.vector.tensor_scalar_add(ve[:], ve[:], EPS)
            sd = small.tile([2, CCH], F, tag="gn_sd")
            nc.scalar.sqrt(sd[:], ve[:])
            r0 = small.tile([2, CCH], F, tag="gn_r0")
            nc.vector.reciprocal(r0[:], sd[:])
            for _ in range(2):
                t0 = small.tile([2, CCH], F, tag="gn_t0")
                nc.vector.tensor_mul(t0[:], r0[:], r0[:])
                nc.vector.tensor_mul(t0[:], t0[:], ve[:])
                nc.vector.tensor_scalar(
                    out=t0[:], in0=t0[:], scalar1=-0.5, scalar2=1.5,
                    op0=OP.mult, op1=OP.add)
                nc.vector.tensor_mul(r0[:], r0[:], t0[:])
            # broadcast (rstd, mean) back to all channels in one matmul
            rs = small.tile([2, CCH, 2], R, tag="gn_rs")
            nc.vector.tensor_copy(rs[:, :, 0], r0[:])
            nc.vector.tensor_copy(rs[:, :, 1], gs[:, :, 0])
            rm_ps = ps_gn.tile([128, 2 * CCH], F, tag="gn_ps")
            nc.tensor.matmul(rm_ps[:], bmask[:],
                             rs[:].rearrange("p c s -> p (c s)"),
                             start=True, stop=True)
            rm = small.tile([128, CCH, 2], F, tag="gn_rm")
            nc.vector.tensor_copy(rm[:].rearrange("p c s -> p (c s)"), rm_ps[:])
            # per-channel affine: xn = x*sc + oc
            sc = small.tile([128, CCH], F, tag="gn_sc")
            nc.vector.tensor_mul(sc[:], rm[:, :, 0], gamma_sb[:])
            oc = small.tile([128, CCH], F, tag="gn_oc")
            nc.vector.tensor_mul(oc[:], rm[:, :, 1], sc[:])
            nc.vector.tensor_sub(oc[:], beta_sb[:], oc[:])
            for c in range(CCH):
                nc.vector.tensor_scalar(
                    out=xn_sb[:, c, :], in0=x_sb[:, c, :],
                    scalar1=sc[:, c:c + 1], scalar2=oc[:, c:c + 1],
                    op0=OP.mult, op1=OP.add)

        # ================= QKV (+ pair-0 scores interleaved) ============
        e_tiles = {}
        av_tiles = {}

        def scores_pair_tt(p, tt):
            # 4 matmuls for head pair (2p, 2p+1), row groups 0/64 concurrent
            if tt == 0:
                for par in (0, 1):
                    e_sb = epool.tile([128, TT, S], BF, tag="e",
                                      name=f"e_{2 * p + par}")
                    e_tiles[2 * p + par] = e_sb
            pss = []
            for par in (0, 1):
                pst = ps_sc_pool.tile([128, S], F, tag="sc",
                                      name=f"sc_{p}_{tt}_{par}")
                pss.append(pst)
            for sh in range(SH):
                for par in (0, 1):
                    psl = slice(64 * par, 64 * par + 64)
                    nc.tensor.matmul(
                        pss[par][:, sh * NS:(sh + 1) * NS],
                        qk_sb[psl, 4 + p, tt * 128:(tt + 1) * 128],
                        qk_sb[psl, p, sh * NS:(sh + 1) * NS],
                        start=True, stop=True, tile_position=(64 * par, 0))
            for par in (0, 1):
                nc.scalar.activation(out=e_tiles[2 * p + par][:, tt, :],
                                     in_=pss[par][:], func=AF.Exp)

        with tc.tile_pool(name="ps_mm", bufs=4, space="PSUM") as ps_mm:
            def qk_chunk(j):  # output chunk j: q for 0-3, k for 4-7
                pss = []
                for sh in range(SH):
                    pst = ps_mm.tile([128, NS], F, tag="mm", name=f"mm_{j}_{sh}")
                    pss.append(pst)
                for c in range(CCH):
                    for sh in range(SH):  # stationary reused across halves
                        nc.tensor.matmul(
                            pss[sh][:], wq_sb[:, c, j * 128:(j + 1) * 128],
                            xn_sb[:, c, sh * NS:(sh + 1) * NS],
                            start=(c == 0), stop=(c == CCH - 1))
                for sh in range(SH):
                    nc.vector.tensor_copy(
                        qk_sb[:, j, sh * NS:(sh + 1) * NS], pss[sh][:])

            def v_chunk(t):  # vT chunk t: [128 seq, 512 channels]
                ps = ps_mm.tile([128, NS], F, tag="mm", name=f"mmv_{t}")
                for c in range(CCH):
                    nc.tensor.matmul(
                        ps[:], xn_sb[:, c, t * 128:(t + 1) * 128],
                        wq_sb[:, c, 2 * C:3 * C],
                        start=(c == 0), stop=(c == CCH - 1))
                nc.vector.tensor_copy(
                    vT_sb[:, t, :, 0:D],
                    ps[:].rearrange("p (h d) -> p h d", h=H))

            qk_chunk(0)
            qk_chunk(4)
            for t in range(TT):
                scores_pair_tt(0, t)
                v_chunk(t)
            for j in (1, 5, 2, 6, 3, 7):
                qk_chunk(j)

        # output-projection weights only needed at the tail
        nc.sync.dma_start(out=wo_sb, in_=woutT_d.rearrange("(c p) o -> p c o", p=128))

        # =========================== Attention ==========================
        att_sb = const.tile([128, CCH, S], R, tag="xn_att", name="att_sb")

        with tc.tile_pool(name="ps_av", bufs=2, space="PSUM") as ps_av_pool:

            def av2(h, t):
                p = h // 2
                if t == 0:
                    if p == H // 2 - 1:
                        # last pair: scores pool is idle by now
                        pst = ps_sc_pool.tile([D + 1, S], F, tag="sc",
                                              name=f"av_{h}")
                    else:
                        pst = ps_av_pool.tile([D + 1, S], F, tag="av",
                                              name=f"av_{h}")
                    av_tiles[h] = pst
                ps_av = av_tiles[h]
                e_sb = e_tiles[h]
                for sh in range(SH):
                    nc.tensor.matmul(
                        ps_av[:, sh * NS:(sh + 1) * NS],
                        vT_sb[:, t, h, :], e_sb[:, t, sh * NS:(sh + 1) * NS],
                        start=(t == 0), stop=(t == TT - 1))

            fin_state = {}

            def fin_pre(h):
                ps_av = av_tiles[h]
                e_tiles.pop(h, None)
                # denominator row -> [64, S/64] -> reciprocal -> row
                den_row = rpool.tile([1, S], F, tag="denrow", name=f"dr_{h}")
                nc.vector.tensor_copy(den_row[:], ps_av[D:D + 1, :])
                denT = rpool.tile([64, S // 64], F, tag="denT", name=f"dt_{h}")
                nc.sync.dma_start(out=denT, in_=den_row)
                rdenT = rpool.tile([64, S // 64], R, tag="rdenT", name=f"rt_{h}")
                with nc.allow_low_precision(reason="float32r is bitwise fp32"):
                    nc.vector.reciprocal(rdenT[:], denT[:])
                rden_d = dpool.tile([1, S], R, tag="rdend", name=f"rdd_{h}")
                nc.sync.dma_start(out=rden_d, in_=rdenT)
                # replicate the reciprocal row to 64 partitions (DRAM
                # APs allow a zero-step partition broadcast)
                rb_sb = rpool.tile([64, S], R, tag="rb", name=f"rb_{h}")
                import concourse.bass as bass_mod
                rden_bcast = bass_mod.AP(
                    tensor=rden_d.tensor, offset=rden_d[:].offset,
                    ap=[[0, 64]] + rden_d[:].ap[1:])
                nc.sync.dma_start(out=rb_sb, in_=rden_bcast)
                fin_state[h] = rb_sb

            def fin_post(h):
                hc, p0 = h // 2, 64 * (h % 2)
                ps_av = av_tiles.pop(h)
                rb_sb = fin_state.pop(h)
                dst = att_sb[p0:p0 + 64, hc, :]
                nc.vector.tensor_mul(dst, ps_av[0:D, :], rb_sb[:])

            # Per pair-iteration: head 2p drains in the first half at double
            # rate, head 2p+1 in the second half, so each fin chain overlaps
            # the following scores instead of blocking the PE stream.
            pending_post = None
            for p in range(H // 2):
                for tt in range(TT):
                    if p + 1 < H // 2:
                        scores_pair_tt(p + 1, tt)
                    if pending_post is not None and tt == 2:
                        fin_post(pending_post)
                        pending_post = None
                    if tt < TT // 2:
                        av2(2 * p, 2 * tt)
                        av2(2 * p, 2 * tt + 1)
                    else:
                        av2(2 * p + 1, 2 * (tt - TT // 2))
                        av2(2 * p + 1, 2 * (tt - TT // 2) + 1)
                    if tt == 4:
                        fin_pre(2 * p)
                    if tt == 6:
                        fin_post(2 * p)
                    if p == 1 and tt == 7:
                        for cc in range(CCH):
                            # fold output bias into the residual late
                            nc.vector.tensor_scalar_add(
                                x_sb[:, cc, :], x_sb[:, cc, :],
                                bout_sb[:, cc:cc + 1])
                fin_pre(2 * p + 1)
                pending_post = 2 * p + 1

            # tail: partial output projection (c<3) overlaps the last fin
            op_tiles = []
            for j in range(CCH):
                if j < 2:
                    pst = ps_av_pool.tile([128, S], F, tag="av", name=f"op_{j}")
                else:
                    pst = ps_sc_pool.tile([128, S], F, tag="sc", name=f"op_{j}")
                op_tiles.append(pst)
                for c in range(CCH - 1):
                    for sh in range(SH):
                        nc.tensor.matmul(
                            pst[:, sh * NS:(sh + 1) * NS],
                            wo_sb[:, c, j * 128:(j + 1) * 128],
                            att_sb[:, c, sh * NS:(sh + 1) * NS],
                            start=(c == 0), stop=False)
                if j == 1 and pending_post is not None:
                    fin_post(pending_post)
                    pending_post = None
            for j in range(CCH):
                pst = op_tiles[j]
                c = CCH - 1
                for sh in range(SH):
                    nc.tensor.matmul(
                        pst[:, sh * NS:(sh + 1) * NS],
                        wo_sb[:, c, j * 128:(j + 1) * 128],
                        att_sb[:, c, sh * NS:(sh + 1) * NS],
                        start=False, stop=True)
                for sh in range(SH):
                    ot = evac.tile([128, NS], R, tag="ot", name=f"ot_{j}_{sh}")
                    nc.vector.tensor_add(
                        ot[:], pst[:, sh * NS:(sh + 1) * NS],
                        x_sb[:, j, sh * NS:(sh + 1) * NS])
                    nc.sync.dma_start(
                        out=out_d.rearrange("(c p) s -> p c s", p=128)
                            [:, j, sh * NS:(sh + 1) * NS],
                        in_=ot[:])



def kernel(x, gamma, beta, w_qkv, w_out, b_out):
    from concourse.bass_utils import run_bass_kernel_spmd

    if "nc" not in _CACHE:
        _CACHE["nc"] = _build()
    nc = _CACHE["nc"]

    x = np.ascontiguousarray(x, dtype=np.float32)
    # host-side layout prep: transpose weights for [K=channel] matmuls and
    # fold the 1/sqrt(D) score scale into w_q
    wqkvT = np.ascontiguousarray(np.asarray(w_qkv).T, dtype=np.float32).copy()
    wqkvT[:, 0:C] *= np.float32(1.0 / np.sqrt(D))
    woutT = np.ascontiguousarray(np.asarray(w_out).T, dtype=np.float32)
    gamma = np.ascontiguousarray(gamma, dtype=np.float32)
    beta = np.ascontiguousarray(beta, dtype=np.float32)
    b_out = np.ascontiguousarray(b_out, dtype=np.float32)

    gmask = np.zeros((128, 2), dtype=np.float32)
    gmask[0:64, 0] = 1.0 / 64
    gmask[64:128, 1] = 1.0 / 64
    bmask = np.zeros((2, 128), dtype=np.float32)
    bmask[0, 0:64] = 1.0
    bmask[1, 64:128] = 1.0
    import ml_dtypes
    onesr = np.ones((1, 64), dtype=np.float32)
    ident = np.eye(128, dtype=np.float32)
    onesv = np.ones((128, 64), dtype=ml_dtypes.bfloat16)
    in_maps = [
        {"x": x[b], "wqkvT": wqkvT, "woutT": woutT,
         "gamma": gamma, "beta": beta, "bout": b_out,
         "gmask": gmask, "bmask": bmask, "onesr": onesr, "onesv": onesv,
         "ident": ident}
        for b in range(B)
    ]
    res = run_bass_kernel_spmd(nc, in_maps, core_ids=list(range(B)), trace=False)
    return np.stack([res.results[b]["out"] for b in range(B)], axis=0)


# revision 22
# speedup vs baseline: 1.2508x; 1.0733x over previous
"""AttentionBlock1D Trainium2 kernel.

Reference computation (per batch element b):
    xn   = GroupNorm(x, 8 groups, eps=1e-5) * gamma + beta
    qkv  = w_qkv @ xn                  # [3C, S], 1x1 conv == channel matmul
    q,k,v split; heads H=8, D=64
    att  = softmax(q^T k / sqrt(D)) v  # per head
    out  = w_out @ att + b_out + x

Distribution: pure data parallelism — batch B=8, one batch element per
NeuronCore (8 cores). Weights replicated.

Per-core dataflow (matmuls in float32r [tf32-like, full PE rate at
N>=256]; the AV stage runs bf16; fp32 PSUM accumulation everywhere):
    - GroupNorm stats via bn_stats/bn_aggr per channel; group reduce and
      broadcast-back are tiny masked matmuls (cross-partition on PE).
    - QKV keeps q,k in [channel, seq] layout; v is produced transposed
      ([seq, channel]) by swapping matmul operands, with a ones column
      appended per head (softmax denominator trick).
    - scores^T[t,s] = (k_h)^T q_h computed directly in [t, s] layout so
      softmax's exp reads PSUM and the AV matmul needs no transposes.
      Max-subtraction is skipped (scores are O(5); exp safe in fp32),
      mathematically identical to jax.nn.softmax.
    - exp on ScalarE (critical engine: S*S*H = 8.4M activations),
      emitted two heads ahead of the AV consumer.
    - AV accumulates [d, s] plus a denominator row over t chunks; the
      normalization multiplies by a K=1 broadcast matmul of the
      reciprocal denominator during PSUM evacuation.
    - output projection accumulates over channel chunks and adds
      (x + b_out) during evacuation.
"""

import numpy as np

B, C, S = 8, 512, 1024
H = 8            # heads
D = C // H       # 64 head dim
EPS = 1e-5
CCH = C // 128   # 4 channel chunks of 128
TT = S // 128    # 8 t (key) chunks of 128
NS = 512         # matmul moving free dim
SH = S // NS     # 2 s halves

_CACHE = {}


def _build(e_bufs=3):
    import concourse.bacc as bacc
    import concourse.tile as tile
    import concourse.mybir as mybir

    nc = bacc.Bacc("TRN2", target_bir_lowering=False, debug=False)
    R = mybir.dt.float32r
    F = mybir.dt.float32

    x_d = nc.dram_tensor("x", [C, S], R, kind="ExternalInput").ap()
    wqkvT_d = nc.dram_tensor("wqkvT", [C, 3 * C], mybir.dt.bfloat16, kind="ExternalInput").ap()
    woutT_d = nc.dram_tensor("woutT", [C, C], R, kind="ExternalInput").ap()
    gamma_d = nc.dram_tensor("gamma", [C], F, kind="ExternalInput").ap()
    beta_d = nc.dram_tensor("beta", [C], F, kind="ExternalInput").ap()
    bout_d = nc.dram_tensor("bout", [C], F, kind="ExternalInput").ap()
    gmask_d = nc.dram_tensor("gmask", [128, 2], R, kind="ExternalInput").ap()
    bmask_d = nc.dram_tensor("bmask", [2, 128], R, kind="ExternalInput").ap()
    onesr_d = nc.dram_tensor("onesr", [1, 64], R, kind="ExternalInput").ap()
    ident_d = nc.dram_tensor("ident", [128, 128], R, kind="ExternalInput").ap()
    onesv_d = nc.dram_tensor("onesv", [128, 64], mybir.dt.bfloat16,
                             kind="ExternalInput").ap()
    out_d = nc.dram_tensor("out", [C, S], R, kind="ExternalOutput").ap()

    with tile.TileContext(nc) as tc:
        _body(tc, nc, mybir, x_d, wqkvT_d, woutT_d, gamma_d, beta_d, bout_d,
              gmask_d, bmask_d, onesr_d, onesv_d, ident_d, out_d, e_bufs)
    nc.compile()
    return nc


def _body(tc, nc, mybir, x_d, wqkvT_d, woutT_d, gamma_d, beta_d, bout_d,
          gmask_d, bmask_d, onesr_d, onesv_d, ident_d, out_d, e_bufs):
    from contextlib import ExitStack

    R = mybir.dt.float32r
    F = mybir.dt.float32
    BF = mybir.dt.bfloat16
    OP = mybir.AluOpType
    AF = mybir.ActivationFunctionType

    ctx = ExitStack()
    with ctx:
        const = ctx.enter_context(tc.tile_pool(name="const", bufs=1))
        small = ctx.enter_context(tc.tile_pool(name="small", bufs=4))
        rpool = ctx.enter_context(tc.tile_pool(name="rpool", bufs=2))
        dpool = ctx.enter_context(tc.tile_pool(name="dpool", bufs=2, space="DRAM"))
        epool = ctx.enter_context(tc.tile_pool(name="epool", bufs=4))
        evac = ctx.enter_context(tc.tile_pool(name="evac", bufs=2))

        # ---- resident tensors ----
        x_sb = const.tile([128, CCH, S], R, tag="x")        # x, later x + b_out
        wq_sb = const.tile([128, CCH, 3 * C], BF, tag="wq")
        wo_sb = const.tile([128, CCH, C], R, tag="wo")
        gamma_sb = const.tile([128, CCH], F, tag="gam")
        beta_sb = const.tile([128, CCH], F, tag="bet")
        bout_sb = const.tile([128, CCH], F, tag="bou")
        qk_sb = const.tile([128, 2 * CCH, S], BF, tag="qk")  # q: 0-3, k: 4-7
        vT_sb = const.tile([128, TT, H, D + 1], BF, tag="vt")  # +ones column
        # xn shares space with att (xn dead after QKV, att written after)
        xn_sb = const.tile([128, CCH, S], BF, tag="xn_att")
        att_sb = None  # allocated later from the same tag

        for c in range(CCH):
            nc.sync.dma_start(out=x_sb[:, c, :],
                              in_=x_d.rearrange("(c p) s -> p c s", p=128)[:, c, :])
        nc.sync.dma_start(out=gamma_sb, in_=gamma_d.rearrange("(c p) -> p c", p=128))
        nc.sync.dma_start(out=beta_sb, in_=beta_d.rearrange("(c p) -> p c", p=128))
        nc.sync.dma_start(out=bout_sb, in_=bout_d.rearrange("(c p) -> p c", p=128))
        for c in range(CCH):
            nc.sync.dma_start(out=wq_sb[:, c, :],
                              in_=wqkvT_d.rearrange("(c p) o -> p c o", p=128)[:, c, :])

        gmask = const.tile([128, 2], R, tag="gmask")   # [channel, group] 1/64
        bmask = const.tile([2, 128], R, tag="bmask")   # [group, channel] 1
        ones_row = const.tile([1, 64], R, tag="ones")  # K=1 broadcast stationary
        ident_sb = const.tile([128, 128], R, tag="ident")
        nc.sync.dma_start(out=gmask, in_=gmask_d)
        nc.sync.dma_start(out=bmask, in_=bmask_d)
        nc.sync.dma_start(out=ones_row, in_=onesr_d)
        nc.sync.dma_start(out=ident_sb, in_=ident_d)
        nc.sync.dma_start(
            out=vT_sb[:, :, :, D:D + 1],
            in_=onesv_d.rearrange("p (t h) -> p t h", t=TT)[:, :, :, None])

        # scores/exp psum pool lives from the start (banks 0-3, LIFO inside)
        ps_sc_pool = ctx.enter_context(
            tc.tile_pool(name="ps_sc", bufs=2, space="PSUM"))

        # =========================== GroupNorm ===========================
        # Stage-batched across the 4 channel chunks to minimize
        # cross-engine ping-pong: one group-reduce matmul and one
        # broadcast-back matmul handle all 8 groups at once.
        with tc.tile_pool(name="ps_gn", bufs=2, space="PSUM") as ps_gn:
            st = small.tile([128, CCH, 2, 6], F, tag="gn_st")
            for c in range(CCH):
                nc.vector.bn_stats(st[:, c, 0, :], x_sb[:, c, 0:512])
                nc.vector.bn_stats(st[:, c, 1, :], x_sb[:, c, 512:1024])
            mv = small.tile([128, CCH, 2], F, tag="gn_mv")  # per-ch mean, var
            for c in range(CCH):
                nc.vector.bn_aggr(mv[:, c, :], st[:, c, :, :])
            ms = small.tile([128, CCH, 2], R, tag="gn_ms")  # mean, E[x^2]
            nc.vector.tensor_copy(ms[:, :, 0:1], mv[:, :, 0:1])
            nc.vector.tensor_mul(ms[:, :, 1:2], mv[:, :, 0:1], mv[:, :, 0:1])
            nc.vector.tensor_add(ms[:, :, 1:2], ms[:, :, 1:2], mv[:, :, 1:2])
            # one group reduce for all chunks: [2, (chunk, stat)]
            gs_ps = ps_gn.tile([2, 2 * CCH], F, tag="gn_ps")
            nc.tensor.matmul(gs_ps[:], gmask[:],
                             ms[:].rearrange("p c s -> p (c s)"),
                             start=True, stop=True)
            gs = small.tile([2, CCH, 2], F, tag="gn_gs")
            nc.vector.tensor_copy(gs[:].rearrange("p c s -> p (c s)"), gs_ps[:])
            # ve = var + eps ; rstd = rsqrt(ve) with 2 Newton steps
            ve = small.tile([2, CCH], F, tag="gn_ve")
            nc.vector.tensor_mul(ve[:], gs[:, :, 0], gs[:, :, 0])
            nc.vector.tensor_sub(ve[:], gs[:, :, 1], ve[:])
            nc.vector.tensor_scalar_add(ve[:], ve[:], EPS)
            sd = small.tile([2, CCH], F, tag="gn_sd")
            nc.scalar.sqrt(sd[:], ve[:])
            r0 = small.tile([2, CCH], F, tag="gn_r0")
            nc.vector.reciprocal(r0[:], sd[:])
            for _ in range(2):
                t0 = small.tile([2, CCH], F, tag="gn_t0")
                nc.vector.tensor_mul(t0[:], r0[:], r0[:])
                nc.vector.tensor_mul(t0[:], t0[:], ve[:])
                nc.vector.tensor_scalar(
                    out=t0[:], in0=t0[:], scalar1=-0.5, scalar2=1.5,
                    op0=OP.mult, op1=OP.add)
                nc.vector.tensor_mul(r0[:], r0[:], t0[:])
            # broadcast (rstd, mean) back to all channels in one matmul
            rs = small.tile([2, CCH, 2], R, tag="gn_rs")
            nc.vector.tensor_copy(rs[:, :, 0], r0[:])
            nc.vector.tensor_copy(rs[:, :, 1], gs[:, :, 0])
            rm_ps = ps_gn.tile([128, 2 * CCH], F, tag="gn_ps")
            nc.tensor.matmul(rm_ps[:], bmask[:],
                             rs[:].rearrange("p c s -> p (c s)"),
                             start=True, stop=True)
            rm = small.tile([128, CCH, 2], F, tag="gn_rm")
            nc.vector.tensor_copy(rm[:].rearrange("p c s -> p (c s)"), rm_ps[:])
            # per-channel affine: xn = x*sc + oc
            sc = small.tile([128, CCH], F, tag="gn_sc")
            nc.vector.tensor_mul(sc[:], rm[:, :, 0], gamma_sb[:])
            oc = small.tile([128, CCH], F, tag="gn_oc")
            nc.vector.tensor_mul(oc[:], rm[:, :, 1], sc[:])
            nc.vector.tensor_sub(oc[:], beta_sb[:], oc[:])
            for c in range(CCH):
                nc.vector.tensor_scalar(
                    out=xn_sb[:, c, :], in0=x_sb[:, c, :],
                    scalar1=sc[:, c:c + 1], scalar2=oc[:, c:c + 1],
                    op0=OP.mult, op1=OP.add)

        # ================= QKV (+ pair-0 scores interleaved) ============
        e_tiles = {}
        av_tiles = {}

        def scores_pair_tt(p, tt):
            # 4 matmuls for head pair (2p, 2p+1), row groups 0/64 concurrent
            if tt == 0:
                for par in (0, 1):
                    e_sb = epool.tile([128, TT, S], BF, tag="e",
                                      name=f"e_{2 * p + par}")
                    e_tiles[2 * p + par] = e_sb
            pss = []
            for par in (0, 1):
                pst = ps_sc_pool.tile([128, S], F, tag="sc",
                                      name=f"sc_{p}_{tt}_{par}")
                pss.append(pst)
            for sh in range(SH):
                for par in (0, 1):
                    psl = slice(64 * par, 64 * par + 64)
                    nc.tensor.matmul(
                        pss[par][:, sh * NS:(sh + 1) * NS],
                        qk_sb[psl, 4 + p, tt * 128:(tt + 1) * 128],
                        qk_sb[psl, p, sh * NS:(sh + 1) * NS],
                        start=True, stop=True, tile_position=(64 * par, 0))
            for par in (0, 1):
                nc.scalar.activation(out=e_tiles[2 * p + par][:, tt, :],
                                     in_=pss[par][:], func=AF.Exp)

        with tc.tile_pool(name="ps_mm", bufs=4, space="PSUM") as ps_mm:
            def qk_chunk(j):  # output chunk j: q for 0-3, k for 4-7
                pss = []
                for sh in range(SH):
                    pst = ps_mm.tile([128, NS], F, tag="mm", name=f"mm_{j}_{sh}")
                    pss.append(pst)
                for c in range(CCH):
                    for sh in range(SH):  # stationary reused across halves
                        nc.tensor.matmul(
                            pss[sh][:], wq_sb[:, c, j * 128:(j + 1) * 128],
                            xn_sb[:, c, sh * NS:(sh + 1) * NS],
                            start=(c == 0), stop=(c == CCH - 1))
                for sh in range(SH):
                    nc.vector.tensor_copy(
                        qk_sb[:, j, sh * NS:(sh + 1) * NS], pss[sh][:])

            def v_chunk(t):  # vT chunk t: [128 seq, 512 channels]
                ps = ps_mm.tile([128, NS], F, tag="mm", name=f"mmv_{t}")
                for c in range(CCH):
                    nc.tensor.matmul(
                        ps[:], xn_sb[:, c, t * 128:(t + 1) * 128],
                        wq_sb[:, c, 2 * C:3 * C],
                        start=(c == 0), stop=(c == CCH - 1))
                nc.vector.tensor_copy(
                    vT_sb[:, t, :, 0:D],
                    ps[:].rearrange("p (h d) -> p h d", h=H))

            qk_chunk(0)
            qk_chunk(4)
            for t in range(TT):
                scores_pair_tt(0, t)
                v_chunk(t)
            for j in (1, 5, 2, 6, 3, 7):
                qk_chunk(j)

        # output-projection weights only needed at the tail
        nc.sync.dma_start(out=wo_sb, in_=woutT_d.rearrange("(c p) o -> p c o", p=128))

        # =========================== Attention ==========================
        att_sb = const.tile([128, CCH, S], R, tag="xn_att", name="att_sb")

        with tc.tile_pool(name="ps_av", bufs=2, space="PSUM") as ps_av_pool:

            def av2(h, t):
                p = h // 2
                if t == 0:
                    if p == H // 2 - 1:
                        # last pair: scores pool is idle by now
                        pst = ps_sc_pool.tile([D + 1, S], F, tag="sc",
                                              name=f"av_{h}")
                    else:
                        pst = ps_av_pool.tile([D + 1, S], F, tag="av",
                                              name=f"av_{h}")
                    av_tiles[h] = pst
                ps_av = av_tiles[h]
                e_sb = e_tiles[h]
                for sh in range(SH):
                    nc.tensor.matmul(
                        ps_av[:, sh * NS:(sh + 1) * NS],
                        vT_sb[:, t, h, :], e_sb[:, t, sh * NS:(sh + 1) * NS],
                        start=(t == 0), stop=(t == TT - 1))

            fin_state = {}

            def fin_pre(h):
                ps_av = av_tiles[h]
                e_tiles.pop(h, None)
                # denominator row -> [64, S/64] -> reciprocal -> row
                den_row = rpool.tile([1, S], F, tag="denrow", name=f"dr_{h}")
                nc.vector.tensor_copy(den_row[:], ps_av[D:D + 1, :])
                denT = rpool.tile([64, S // 64], F, tag="denT", name=f"dt_{h}")
                nc.sync.dma_start(out=denT, in_=den_row)
                rdenT = rpool.tile([64, S // 64], R, tag="rdenT", name=f"rt_{h}")
                with nc.allow_low_precision(reason="float32r is bitwise fp32"):
                    nc.vector.reciprocal(rdenT[:], denT[:])
                rden_d = dpool.tile([1, S], R, tag="rdend", name=f"rdd_{h}")
                nc.sync.dma_start(out=rden_d, in_=rdenT)
                # replicate the reciprocal row to 64 partitions (DRAM
                # APs allow a zero-step partition broadcast)
                rb_sb = rpool.tile([64, S], R, tag="rb", name=f"rb_{h}")
                import concourse.bass as bass_mod
                rden_bcast = bass_mod.AP(
                    tensor=rden_d.tensor, offset=rden_d[:].offset,
                    ap=[[0, 64]] + rden_d[:].ap[1:])
                nc.sync.dma_start(out=rb_sb, in_=rden_bcast)
                fin_state[h] = rb_sb

            def fin_post(h):
                hc, p0 = h // 2, 64 * (h % 2)
                ps_av = av_tiles.pop(h)
                rb_sb = fin_state.pop(h)
                dst = att_sb[p0:p0 + 64, hc, :]
                nc.vector.tensor_mul(dst, ps_av[0:D, :], rb_sb[:])

            # Per pair-iteration: head 2p drains in the first half at double
            # rate, head 2p+1 in the second half, so each fin chain overlaps
            # the following scores instead of blocking the PE stream.
            pending_post = None
            for p in range(H // 2):
                for tt in range(TT):
                    if p + 1 < H // 2:
                        scores_pair_tt(p + 1, tt)
                    if pending_post is not None and tt == 2:
                        fin_post(pending_post)
                        pending_post = None
                    if tt < TT // 2:
                        av2(2 * p, 2 * tt)
                        av2(2 * p, 2 * tt + 1)
                    else:
                        av2(2 * p + 1, 2 * (tt - TT // 2))
                        av2(2 * p + 1, 2 * (tt - TT // 2) + 1)
                    if tt == 4:
                        fin_pre(2 * p)
                    if tt == 6:
                        fin_post(2 * p)
                    if p == 1 and tt == 7:
                        for cc in range(CCH):
                            # fold output bias into the residual late
                            nc.vector.tensor_scalar_add(
                                x_sb[:, cc, :], x_sb[:, cc, :],
                                bout_sb[:, cc:cc + 1])
                fin_pre(2 * p + 1)
                pending_post = 2 * p + 1

            # tail: partial output projection (c<3) overlaps the last fin
            op_tiles = []
            for j in range(CCH):
                if j < 2:
                    pst = ps_av_pool.tile([128, S], F, tag="av", name=f"op_{j}")
                else:
                    pst = ps_sc_pool.tile([128, S], F, tag="sc", name=f"op_{j}")
                op_tiles.append(pst)
                for c in range(CCH - 1):
                    for sh in range(SH):
                        nc.tensor.matmul(
                            pst[:, sh * NS:(sh + 1) * NS],
                            wo_sb[:, c, j * 128:(j + 1) * 128],
                            att_sb[:, c, sh * NS:(sh + 1) * NS],
                            start=(c == 0), stop=False)
                if j == 1 and pending_post is not None:
                    fin_post(pending_post)
                    pending_post = None
            for j in range(CCH):
                pst = op_tiles[j]
                c = CCH - 1
                for sh in range(SH):
                    nc.tensor.matmul(
                        pst[:, sh * NS:(sh + 1) * NS],
                        wo_sb[:, c, j * 128:(j + 1) * 128],
                        att_sb[:, c, sh * NS:(sh + 1) * NS],
                        start=False, stop=True)
                for sh in range(SH):
                    ot = evac.tile([128, NS], R, tag="ot", name=f"ot_{j}_{sh}")
                    nc.vector.tensor_add(
                        ot[:], pst[:, sh * NS:(sh + 1) * NS],
                        x_sb[:, j, sh * NS:(sh + 1) * NS])
                    nc.sync.dma_start(
                        out=out_d.rearrange("(c p) s -> p c s", p=128)
                            [:, j, sh * NS:(sh + 1) * NS],
                        in_=ot[:])



def kernel(x, gamma, beta, w_qkv, w_out, b_out):
    from concourse.bass_utils import run_bass_kernel_spmd

    if "nc" not in _CACHE:
        _CACHE["nc"] = _build()
    nc = _CACHE["nc"]

    x = np.ascontiguousarray(x, dtype=np.float32)
    # host-side layout prep: transpose weights for [K=channel] matmuls and
    # fold the 1/sqrt(D) score scale into w_q
    import ml_dtypes
    wqkvT = np.ascontiguousarray(np.asarray(w_qkv).T, dtype=np.float32).copy()
    wqkvT[:, 0:C] *= np.float32(1.0 / np.sqrt(D))
    wqkvT = wqkvT.astype(ml_dtypes.bfloat16)
    woutT = np.ascontiguousarray(np.asarray(w_out).T, dtype=np.float32)
    gamma = np.ascontiguousarray(gamma, dtype=np.float32)
    beta = np.ascontiguousarray(beta, dtype=np.float32)
    b_out = np.ascontiguousarray(b_out, dtype=np.float32)

    gmask = np.zeros((128, 2), dtype=np.float32)
    gmask[0:64, 0] = 1.0 / 64
    gmask[64:128, 1] = 1.0 / 64
    bmask = np.zeros((2, 128), dtype=np.float32)
    bmask[0, 0:64] = 1.0
    bmask[1, 64:128] = 1.0
    onesr = np.ones((1, 64), dtype=np.float32)
    ident = np.eye(128, dtype=np.float32)
    onesv = np.ones((128, 64), dtype=ml_dtypes.bfloat16)
    in_maps = [
        {"x": x[b], "wqkvT": wqkvT, "woutT": woutT,
         "gamma": gamma, "beta": beta, "bout": b_out,
         "gmask": gmask, "bmask": bmask, "onesr": onesr, "onesv": onesv,
         "ident": ident}
        for b in range(B)
    ]
    res = run_bass_kernel_spmd(nc, in_maps, core_ids=list(range(B)), trace=False)
    return np.stack([res.results[b]["out"] for b in range(B)], axis=0)
